# revision 37
# baseline (speedup 1.0000x reference)
"""ChordMixerBlock Trainium2 kernel.

Math (per batch b):
    h   = gelu(data @ w1 + b1)            # exact gelu
    y   = h @ w2 + b2
    out[l, :] = rotate_chord(y)[l, :] + data[l, :]
where rotate_chord rolls track t (channels [16t, 16t+16)) forward by
s_t = 2^(t-1) positions along L (track 0: no shift; track 15: 2^14 == L
-> no shift).

Sharding: 8 cores = (batch b, L-half j); each core computes y for its own
8192-token chunk in transposed layout [256 d, 8192 l] so the contraction
dim D lands on SBUF partitions (host pre-transposes inputs and transposes
the output back).

Roll handling is entirely layout-based -- no cross-core traffic:
  * acc[c, p] = y[c, p] + b2[c] + dataS[c, p], where dataS is the residual
    pre-rolled by +s_t per track on the HOST (pure sharding-layout prep).
    acc[c, p] is then exactly out[global (c0 + p - s_t) mod L, c] -- a
    complete output value, merely stored at a per-track rotated column.
  * Each core dumps acc verbatim; the HOST undoes the per-track column
    rotation while unsharding (np.roll per 16-channel track), so no
    collective and no boundary exchange is needed on device.

Device program per core (pure bf16 data path, fp32 accumulate in PSUM):
  stream dataM/dataS in over the gpsimd/vector DGE rings, then for each
  group of 4 l-tiles (512 cols each): fc1 matmuls ordered so each of the
  8 w1 stationary tiles is loaded once per group (LDWEIGHTS count 4x
  lower than one-load-per-matmul), gelu+bias on the scalar engine, fc2
  likewise with the 8 w2 tiles, then a vector scalar_tensor_tensor adds
  b2 + rolled residual and writes bf16 acc, which streams out on the
  sync ring.  Output is bf16 (the residual dominates the output scale,
  so bf16 rounding stays ~3e-3 relative); the host upcasts to fp32.
"""

import sys

sys.path.insert(0, "/opt/trn_rl_repo")

import numpy as np
import ml_dtypes

import concourse.bass as bass
import concourse.bacc as bacc
import concourse.tile as tile
import concourse.mybir as mybir
from concourse import bass_utils

B, L, D, H = 4, 16384, 256, 512
N_CORES = 8
LC = L // 2                      # per-core chunk length
NT, TS = 16, 16                  # tracks, track size
SHIFTS = [0] + [2 ** i for i in range(NT - 1)]
SEFF = [s % L for s in SHIFTS]   # track 15 -> 0
TILE = 512                       # l-tile width for matmuls
NTILES = LC // TILE              # 16
G = 4                            # l-tiles per weight-reuse group
NGROUPS = NTILES // G            # 4
ISLICE = 1024                    # input DMA slice width

F32 = mybir.dt.float32
BF16 = mybir.dt.bfloat16
F8 = mybir.dt.float8e4


def _build():
    nc = bacc.Bacc(
        "TRN2", target_bir_lowering=False, debug=False,
        num_devices=N_CORES,
    )

    dataM_h = nc.dram_tensor("dataM", [D, LC], BF16, kind="ExternalInput")
    dataS_h = nc.dram_tensor("dataS", [D, LC], BF16, kind="ExternalInput")
    # all bf16 weights in one packed tensor (one DMA descriptor: the input
    # rings are issue-rate limited at ~0.6us per descriptor early on):
    # cols [dt*512:+512] = w1 rows [dt*128:+128]; cols [1024+ht*256:+256] =
    # w2 rows [ht*128:+128].
    # fc2 splits the H contraction: rows 0:256 in bf16, rows 256:512 as an
    # fp8 DoubleRow pack [ki, ko, m] = w2[256 + 128*ko + ki, m] (one matmul
    # contracts all 256 rows; quantization error stays ~1.5e-2 vs the 2e-2
    # budget because only half of fc2 is fp8)
    w1pk_h = nc.dram_tensor("w1pk", [128, 1024], BF16, kind="ExternalInput")
    w2pk_h = nc.dram_tensor("w2pk", [128, 512], BF16, kind="ExternalInput")
    w28_h = nc.dram_tensor("w28", [128, 2, D], F8, kind="ExternalInput")
    bpk_h = nc.dram_tensor("bpk", [128, 6], F32, kind="ExternalInput")
    outT_h = nc.dram_tensor("outT", [D, LC], BF16, kind="ExternalOutput")

    with tile.TileContext(nc) as tc:
        with (
            tc.tile_pool(name="const", bufs=1) as cpool,
            tc.tile_pool(name="big", bufs=1) as big,
            tc.tile_pool(name="hbf", bufs=20) as hbfp,
            tc.tile_pool(name="h8", bufs=10) as h8p,
            tc.tile_pool(name="ph", bufs=4, space="PSUM") as php,
            tc.tile_pool(name="py", bufs=4, space="PSUM") as pyp,
        ):
            # --- weights: w1 pack leads the sync ring (gates the first
            # matmul); w2/fp8/bias packs slot into the gpsimd dm stream
            # just before each is first needed (each ring only sustains
            # ~170 GB/s, so every early byte is critical-path) ---
            w1pk = cpool.tile([128, 1024], BF16, tag="w1pk")
            nc.sync.dma_start(w1pk[:], w1pk_h.ap())
            w2pk = cpool.tile([128, 512], BF16, tag="w2pk")

            def w1s(dt, ht):
                # stationary [128, 128] tile of w1 rows dt*128, cols ht*128
                return w1pk[:, dt * 512 + ht * 128:dt * 512 + ht * 128 + 128]

            def w2s(ht, k):
                o = ht * 256 + k * 128
                return w2pk[:, o:o + 128]

            # --- PE warmup: dependency-free matmuls on scratch SBUF keep
            # the PE-HAM activity window busy while inputs stream in, so
            # the real matmuls start at the full 2.4 GHz clock ---
            wscr = cpool.tile([128, 128], BF16, tag="wscr")
            nc.vector.memset(wscr[:], 0)
            for wi in range(32):
                pw = php.tile([128, TILE], F32, tag="ph", name=f"warm{wi}")
                nc.tensor.matmul(
                    pw[:, 0:128], wscr[:], wscr[:], start=True, stop=True,
                )

            # --- persistent chunk buffers ---
            dm = [big.tile([128, LC], BF16, tag=f"dm{k}", name=f"dm{k}")
                  for k in range(2)]
            ds = [big.tile([128, LC], BF16, tag=f"ds{k}", name=f"ds{k}")
                  for k in range(2)]
            acc = [big.tile([128, LC], BF16, tag=f"acc{k}", name=f"acc{k}")
                   for k in range(2)]

            # inputs: dm k=0 on the sync ring (behind the weight pack),
            # dm k=1 leads the gpsimd ring; first slices small so the
            # first matmuls start early, then wide to keep per-descriptor
            # issue cost low; ds halves follow on the same rings (first
            # needed only at fc2(g0), ~10us later), then the fp8 weight
            # pack and biases (first needed at gelu/fc2 of g0).
            w28sb = cpool.tile([128, 2, D], F8, tag="w28")
            bpk = cpool.tile([128, 6], F32, tag="bpk")
            dm_cuts = [0, 1024, 2048, 4096, 6144, LC]
            for s in range(len(dm_cuts) - 1):
                sl = slice(dm_cuts[s], dm_cuts[s + 1])
                nc.sync.dma_start(dm[0][:, sl], dataM_h.ap()[0:128, sl])
                nc.gpsimd.dma_start(dm[1][:, sl], dataM_h.ap()[128:256, sl])
                if s == 0:
                    # tiny, and gate the first gelu / fc2 DR matmul
                    nc.gpsimd.dma_start(bpk[:], bpk_h.ap())
                    nc.gpsimd.dma_start(w28sb[:, 0:2, :], w28_h.ap())
                if s == 2:
                    # bf16 w2 half, first needed at fc2(g0) ~10us in
                    nc.gpsimd.dma_start(w2pk[:], w2pk_h.ap())
            for s in range(0, LC, 2048):
                sl = slice(s, s + 2048)
                nc.sync.dma_start(ds[0][:, sl], dataS_h.ap()[0:128, sl])
                nc.gpsimd.dma_start(ds[1][:, sl], dataS_h.ap()[128:256, sl])

            # --- main loop: groups of G l-tiles, weight-reuse inside,
            # software-pipelined one group back: fc2(g-1) k-blocks are
            # emitted between fc1(g) ht-blocks so the PE always has
            # matmul work while gelu catches up on ph banks. ---
            hbf = {}
            h8 = {}

            def csl(g, j):
                i = g * G + j
                return slice(i * TILE, (i + 1) * TILE)

            def fc1_block(g, ht):
                # ph[j] = w1[:, ht-slice]^T @ dm over both 128-row halves;
                # each w1 stationary tile loaded once per block.
                ph = [None] * G
                for dt in range(2):
                    for j in range(G):
                        if dt == 0:
                            ph[j] = php.tile([128, TILE], F32, tag="ph",
                                             name=f"ph_{g}_{j}_{ht}")
                        nc.tensor.matmul(
                            ph[j][:], w1s(dt, ht), dm[dt][:, csl(g, j)],
                            start=(dt == 0), stop=(dt == 1),
                        )
                for j in range(G):
                    if ht < 2:
                        hb = hbfp.tile([128, TILE], BF16, tag="hbf",
                                       name=f"hbf_{g}_{j}_{ht}")
                        dst = hb[:]
                        hbf[(g, j, ht)] = hb
                    else:
                        # h rows 256:512 quantize to fp8 for the DoubleRow
                        # half of fc2; plane ko = ht - 2
                        if ht == 2:
                            h8[(g, j)] = h8p.tile([128, 2, TILE], F8,
                                                  tag="h8",
                                                  name=f"h8_{g}_{j}")
                        dst = h8[(g, j)][:, ht - 2, :]
                    nc.scalar.activation(
                        dst, ph[j][:],
                        mybir.ActivationFunctionType.Gelu,
                        bias=bpk[:, ht:ht + 1],
                    )

            def fc2_block(g, k):
                dsl = slice(k * 128, (k + 1) * 128)
                last = g == NGROUPS - 1
                tail = last and k == 1

                def emit_stt(j, py_j):
                    # acc = (y + b2) + rolled residual, bf16 out
                    nc.vector.scalar_tensor_tensor(
                        acc[k][:, csl(g, j)], py_j[:], bpk[:, 4 + k:5 + k],
                        ds[k][:, csl(g, j)],
                        mybir.AluOpType.add, mybir.AluOpType.add,
                    )
                    if last:
                        # final group: drip each finished 512-tile out on
                        # both HWDGE rings (gelus are done, scalar is free)
                        eng = nc.scalar if j % 2 == 0 else nc.sync
                        eng.dma_start(
                            outT_h.ap()[k * 128:(k + 1) * 128, csl(g, j)],
                            acc[k][:, csl(g, j)],
                        )

                if tail:
                    # very last block: finish tile-by-tile so the vector
                    # engine's STT chain overlaps the remaining matmuls
                    # instead of serializing after the last one
                    for j in range(G):
                        py_j = pyp.tile([128, TILE], F32, tag="py",
                                        name=f"py_{g}_{j}_{k}")
                        for ht in range(2):
                            nc.tensor.matmul(
                                py_j[:], w2s(ht, k), hbf[(g, j, ht)][:],
                                start=(ht == 0), stop=False,
                            )
                        nc.tensor.matmul(
                            py_j[:], w28sb[:, :, dsl], h8[(g, j)][:, 0:2, :],
                            start=False, stop=True,
                            perf_mode=mybir.MatmulPerfMode.DoubleRow,
                        )
                        emit_stt(j, py_j)
                    return

                py = [None] * G
                for ht in range(2):
                    for j in range(G):
                        if ht == 0:
                            py[j] = pyp.tile([128, TILE], F32, tag="py",
                                             name=f"py_{g}_{j}_{k}")
                        nc.tensor.matmul(
                            py[j][:], w2s(ht, k), hbf[(g, j, ht)][:],
                            start=(ht == 0), stop=False,
                        )
                for j in range(G):
                    # fp8 DoubleRow: contracts h rows 256:512 in one matmul
                    nc.tensor.matmul(
                        py[j][:], w28sb[:, :, dsl], h8[(g, j)][:, 0:2, :],
                        start=False, stop=True,
                        perf_mode=mybir.MatmulPerfMode.DoubleRow,
                    )
                for j in range(G):
                    emit_stt(j, py[j])

            def out_block(g):
                bsl = slice(g * G * TILE, (g + 1) * G * TILE)
                for k in range(2):
                    nc.sync.dma_start(
                        outT_h.ap()[k * 128:(k + 1) * 128, bsl],
                        acc[k][:, bsl],
                    )

            for g in range(NGROUPS + 1):
                if g < NGROUPS:
                    fc1_block(g, 0)
                    fc1_block(g, 1)
                if g > 0:
                    fc2_block(g - 1, 0)
                if g < NGROUPS:
                    fc1_block(g, 2)
                    fc1_block(g, 3)
                if g > 0:
                    fc2_block(g - 1, 1)
                    if g - 1 < NGROUPS - 1:
                        out_block(g - 1)

    nc.compile()
    return nc


_NC = None


def _get_nc():
    global _NC
    if _NC is None:
        _NC = _build()
    return _NC


def make_in_maps(data, w1, b1, w2, b2):
    data = np.asarray(data, dtype=np.float32)
    w1f = np.asarray(w1, dtype=np.float32)
    w2f = np.asarray(w2, dtype=np.float32)
    # packed bf16 weights: w1pk = [w1 rows 0:128 | w1 rows 128:256],
    # w2pk = [w2 rows 0:128 | w2 rows 128:256]
    w1pk = np.ascontiguousarray(np.concatenate(
        [w1f[0:128, :], w1f[128:256, :]], axis=1,
    )).astype(ml_dtypes.bfloat16)
    w2pk = np.ascontiguousarray(np.concatenate(
        [w2f[0:128, :], w2f[128:256, :]], axis=1,
    )).astype(ml_dtypes.bfloat16)
    # DoubleRow pack: [ki, ko, m] = w2[256 + 128*ko + ki, m]
    w28 = np.ascontiguousarray(
        w2f[H // 2:].reshape(2, 128, D).transpose(1, 0, 2)
    ).astype(ml_dtypes.float8_e4m3)
    # packed biases: cols 0:4 = b1 (col ht = rows ht*128..), cols 4:6 = b2
    bpk = np.ascontiguousarray(np.concatenate(
        [np.asarray(b1, dtype=np.float32).reshape(4, 128).T,
         np.asarray(b2, dtype=np.float32).reshape(2, 128).T], axis=1,
    ))

    in_maps = []
    for bb in range(B):
        # residual pre-rolled by +s_t per track:
        # rolled[l, c] = data[(l - s_t) mod L, c]
        rolled = np.empty((L, D), dtype=np.float32)
        for t in range(NT):
            cs = slice(t * TS, (t + 1) * TS)
            rolled[:, cs] = np.roll(data[bb, :, cs], SEFF[t], axis=0)
        for j in range(2):
            sl = slice(j * LC, (j + 1) * LC)
            dataM = np.ascontiguousarray(
                data[bb, sl, :].T.astype(ml_dtypes.bfloat16)
            )
            dataS = np.ascontiguousarray(
                rolled[sl, :].T.astype(ml_dtypes.bfloat16)
            )
            in_maps.append({
                "dataM": dataM, "dataS": dataS,
                "w1pk": w1pk, "w2pk": w2pk, "w28": w28, "bpk": bpk,
            })
    return in_maps


def kernel(data, w1, b1, w2, b2):
    nc = _get_nc()
    in_maps = make_in_maps(data, w1, b1, w2, b2)
    res = bass_utils.run_bass_kernel_spmd(
        nc, in_maps, core_ids=list(range(N_CORES))
    )
    out = np.empty((B, L, D), dtype=np.float32)
    for bb in range(B):
        # full[c, g] = out[(g - s_t(c)) mod L, c]; undo per-track rotation
        full = np.concatenate(
            [np.asarray(res.results[2 * bb + j]["outT"], dtype=np.float32)
             for j in range(2)], axis=1,
        )
        for t in range(NT):
            seg = full[t * TS:(t + 1) * TS, :]
            out[bb, :, t * TS:(t + 1) * TS] = np.roll(seg, -SEFF[t], axis=1).T
    return out


# revision 40
# speedup vs baseline: 1.0035x; 1.0035x over previous
"""ChordMixerBlock Trainium2 kernel.

Math (per batch b):
    h   = gelu(data @ w1 + b1)            # exact gelu
    y   = h @ w2 + b2
    out[l, :] = rotate_chord(y)[l, :] + data[l, :]
where rotate_chord rolls track t (channels [16t, 16t+16)) forward by
s_t = 2^(t-1) positions along L (track 0: no shift; track 15: 2^14 == L
-> no shift).

Sharding: 8 cores = (batch b, L-half j); each core computes y for its own
8192-token chunk in transposed layout [256 d, 8192 l] so the contraction
dim D lands on SBUF partitions (host pre-transposes inputs and transposes
the output back).

Roll handling is entirely layout-based -- no cross-core traffic:
  * acc[c, p] = y[c, p] + b2[c] + dataS[c, p], where dataS is the residual
    pre-rolled by +s_t per track on the HOST (pure sharding-layout prep).
    acc[c, p] is then exactly out[global (c0 + p - s_t) mod L, c] -- a
    complete output value, merely stored at a per-track rotated column.
  * Each core dumps acc verbatim; the HOST undoes the per-track column
    rotation while unsharding (np.roll per 16-channel track), so no
    collective and no boundary exchange is needed on device.

Device program per core (bf16 data path, fp32 accumulate in PSUM; the
upper half of the fc2 contraction runs as one fp8e4 DoubleRow matmul,
keeping rel err ~1.5e-2 vs the 2e-2 budget):
  * ~28 dependency-free warmup matmuls on a zeroed scratch tile keep the
    PE-HAM activity window busy during the input-DMA wait so real
    matmuls start at the full 2.4 GHz clock.
  * dataM streams k-interleaved on the sync HWDGE ring; bias/fp8/w2
    packs lead the gpsimd ring (they gate gelu / fc2(g0)), followed by
    the rolled residual.  Weights are packed into few descriptors: the
    rings are issue-rate/bandwidth limited (~170 GB/s each) early on.
  * Main loop in groups of 4 l-tiles (512 cols), software-pipelined one
    group back (fc2(g-1) k-blocks between fc1(g) ht-blocks) so the PE
    never waits on the scalar engine's gelu chain for PSUM banks.
  * gelu+bias on the scalar engine: h rows 0:256 -> bf16, rows 256:512
    -> fp8 planes of a [128, 2, 512] tile consumed by the DoubleRow MM.
  * vector scalar_tensor_tensor adds b2 + rolled residual, writes bf16
    acc, streamed out on sync; the last group finishes tile-by-tile on
    both free rings to shorten the tail.  Host upcasts bf16 -> fp32.
"""

import sys

sys.path.insert(0, "/opt/trn_rl_repo")

import numpy as np
import ml_dtypes

import concourse.bass as bass
import concourse.bacc as bacc
import concourse.tile as tile
import concourse.mybir as mybir
from concourse import bass_utils

B, L, D, H = 4, 16384, 256, 512
N_CORES = 8
LC = L // 2                      # per-core chunk length
NT, TS = 16, 16                  # tracks, track size
SHIFTS = [0] + [2 ** i for i in range(NT - 1)]
SEFF = [s % L for s in SHIFTS]   # track 15 -> 0
TILE = 512                       # l-tile width for matmuls
NTILES = LC // TILE              # 16
G = 4                            # l-tiles per weight-reuse group
NGROUPS = NTILES // G            # 4
ISLICE = 1024                    # input DMA slice width

F32 = mybir.dt.float32
BF16 = mybir.dt.bfloat16
F8 = mybir.dt.float8e4


def _build():
    nc = bacc.Bacc(
        "TRN2", target_bir_lowering=False, debug=False,
        num_devices=N_CORES,
    )

    dataM_h = nc.dram_tensor("dataM", [D, LC], BF16, kind="ExternalInput")
    dataS_h = nc.dram_tensor("dataS", [D, LC], BF16, kind="ExternalInput")
    # all bf16 weights in one packed tensor (one DMA descriptor: the input
    # rings are issue-rate limited at ~0.6us per descriptor early on):
    # cols [dt*512:+512] = w1 rows [dt*128:+128]; cols [1024+ht*256:+256] =
    # w2 rows [ht*128:+128].
    # fc2 splits the H contraction: rows 0:256 in bf16, rows 256:512 as an
    # fp8 DoubleRow pack [ki, ko, m] = w2[256 + 128*ko + ki, m] (one matmul
    # contracts all 256 rows; quantization error stays ~1.5e-2 vs the 2e-2
    # budget because only half of fc2 is fp8)
    w1pk_h = nc.dram_tensor("w1pk", [128, 1024], BF16, kind="ExternalInput")
    w2pk_h = nc.dram_tensor("w2pk", [128, 512], BF16, kind="ExternalInput")
    w28_h = nc.dram_tensor("w28", [128, 2, D], F8, kind="ExternalInput")
    bpk_h = nc.dram_tensor("bpk", [128, 6], F32, kind="ExternalInput")
    outT_h = nc.dram_tensor("outT", [D, LC], BF16, kind="ExternalOutput")

    with tile.TileContext(nc) as tc:
        with (
            tc.tile_pool(name="const", bufs=1) as cpool,
            tc.tile_pool(name="big", bufs=1) as big,
            tc.tile_pool(name="hbf", bufs=20) as hbfp,
            tc.tile_pool(name="h8", bufs=10) as h8p,
            tc.tile_pool(name="ph", bufs=4, space="PSUM") as php,
            tc.tile_pool(name="py", bufs=4, space="PSUM") as pyp,
        ):
            # --- weights: w1 pack leads the sync ring (gates the first
            # matmul); w2/fp8/bias packs slot into the gpsimd dm stream
            # just before each is first needed (each ring only sustains
            # ~170 GB/s, so every early byte is critical-path) ---
            w1pk = cpool.tile([128, 1024], BF16, tag="w1pk")
            nc.sync.dma_start(w1pk[:], w1pk_h.ap())
            w2pk = cpool.tile([128, 512], BF16, tag="w2pk")

            def w1s(dt, ht):
                # stationary [128, 128] tile of w1 rows dt*128, cols ht*128
                return w1pk[:, dt * 512 + ht * 128:dt * 512 + ht * 128 + 128]

            def w2s(ht, k):
                o = ht * 256 + k * 128
                return w2pk[:, o:o + 128]

            # --- PE warmup: dependency-free matmuls on scratch SBUF keep
            # the PE-HAM activity window busy while inputs stream in, so
            # the real matmuls start at the full 2.4 GHz clock ---
            wscr = cpool.tile([128, 128], BF16, tag="wscr")
            nc.vector.memset(wscr[:], 0)
            for wi in range(28):
                pw = php.tile([128, TILE], F32, tag="ph", name=f"warm{wi}")
                nc.tensor.matmul(
                    pw[:, 0:128], wscr[:], wscr[:], start=True, stop=True,
                )

            # --- persistent chunk buffers ---
            dm = [big.tile([128, LC], BF16, tag=f"dm{k}", name=f"dm{k}")
                  for k in range(2)]
            ds = [big.tile([128, LC], BF16, tag=f"ds{k}", name=f"ds{k}")
                  for k in range(2)]
            acc = [big.tile([128, LC], BF16, tag=f"acc{k}", name=f"acc{k}")
                   for k in range(2)]

            # inputs: dm k=0 on the sync ring (behind the weight pack),
            # dm k=1 leads the gpsimd ring; first slices small so the
            # first matmuls start early, then wide to keep per-descriptor
            # issue cost low; ds halves follow on the same rings (first
            # needed only at fc2(g0), ~10us later), then the fp8 weight
            # pack and biases (first needed at gelu/fc2 of g0).
            # dm: both halves k-interleaved on the sync HWDGE ring (the
            # measured-best arrangement); gpsimd: tiny bias/fp8/w2 packs
            # first (they gate gelu and fc2(g0)), then the residual.
            w28sb = cpool.tile([128, 2, D], F8, tag="w28")
            bpk = cpool.tile([128, 6], F32, tag="bpk")
            nc.gpsimd.dma_start(bpk[:], bpk_h.ap())
            nc.gpsimd.dma_start(w28sb[:, 0:2, :], w28_h.ap())
            nc.gpsimd.dma_start(w2pk[:], w2pk_h.ap())
            for s in range(0, LC, ISLICE):
                sl = slice(s, s + ISLICE)
                for k in range(2):
                    nc.sync.dma_start(
                        dm[k][:, sl], dataM_h.ap()[k * 128:(k + 1) * 128, sl])
            for s in range(0, LC, 2048):
                sl = slice(s, s + 2048)
                for k in range(2):
                    nc.gpsimd.dma_start(
                        ds[k][:, sl], dataS_h.ap()[k * 128:(k + 1) * 128, sl])

            # --- main loop: groups of G l-tiles, weight-reuse inside,
            # software-pipelined one group back: fc2(g-1) k-blocks are
            # emitted between fc1(g) ht-blocks so the PE always has
            # matmul work while gelu catches up on ph banks. ---
            hbf = {}
            h8 = {}

            def csl(g, j):
                i = g * G + j
                return slice(i * TILE, (i + 1) * TILE)

            def fc1_block(g, ht):
                # ph[j] = w1[:, ht-slice]^T @ dm over both 128-row halves;
                # each w1 stationary tile loaded once per block.
                ph = [None] * G
                for dt in range(2):
                    for j in range(G):
                        if dt == 0:
                            ph[j] = php.tile([128, TILE], F32, tag="ph",
                                             name=f"ph_{g}_{j}_{ht}")
                        nc.tensor.matmul(
                            ph[j][:], w1s(dt, ht), dm[dt][:, csl(g, j)],
                            start=(dt == 0), stop=(dt == 1),
                        )
                for j in range(G):
                    if ht < 2:
                        hb = hbfp.tile([128, TILE], BF16, tag="hbf",
                                       name=f"hbf_{g}_{j}_{ht}")
                        dst = hb[:]
                        hbf[(g, j, ht)] = hb
                    else:
                        # h rows 256:512 quantize to fp8 for the DoubleRow
                        # half of fc2; plane ko = ht - 2
                        if ht == 2:
                            h8[(g, j)] = h8p.tile([128, 2, TILE], F8,
                                                  tag="h8",
                                                  name=f"h8_{g}_{j}")
                        dst = h8[(g, j)][:, ht - 2, :]
                    nc.scalar.activation(
                        dst, ph[j][:],
                        mybir.ActivationFunctionType.Gelu,
                        bias=bpk[:, ht:ht + 1],
                    )

            def fc2_block(g, k):
                dsl = slice(k * 128, (k + 1) * 128)
                last = g == NGROUPS - 1
                tail = last and k == 1

                def emit_stt(j, py_j):
                    # acc = (y + b2) + rolled residual, bf16 out
                    nc.vector.scalar_tensor_tensor(
                        acc[k][:, csl(g, j)], py_j[:], bpk[:, 4 + k:5 + k],
                        ds[k][:, csl(g, j)],
                        mybir.AluOpType.add, mybir.AluOpType.add,
                    )
                    if last:
                        # final group: drip each finished 512-tile out on
                        # both HWDGE rings (gelus are done, scalar is free)
                        eng = nc.scalar if j % 2 == 0 else nc.sync
                        eng.dma_start(
                            outT_h.ap()[k * 128:(k + 1) * 128, csl(g, j)],
                            acc[k][:, csl(g, j)],
                        )

                if tail:
                    # very last block: finish tile-by-tile so the vector
                    # engine's STT chain overlaps the remaining matmuls
                    # instead of serializing after the last one
                    for j in range(G):
                        py_j = pyp.tile([128, TILE], F32, tag="py",
                                        name=f"py_{g}_{j}_{k}")
                        for ht in range(2):
                            nc.tensor.matmul(
                                py_j[:], w2s(ht, k), hbf[(g, j, ht)][:],
                                start=(ht == 0), stop=False,
                            )
                        nc.tensor.matmul(
                            py_j[:], w28sb[:, :, dsl], h8[(g, j)][:, 0:2, :],
                            start=False, stop=True,
                            perf_mode=mybir.MatmulPerfMode.DoubleRow,
                        )
                        emit_stt(j, py_j)
                    return

                py = [None] * G
                for ht in range(2):
                    for j in range(G):
                        if ht == 0:
                            py[j] = pyp.tile([128, TILE], F32, tag="py",
                                             name=f"py_{g}_{j}_{k}")
                        nc.tensor.matmul(
                            py[j][:], w2s(ht, k), hbf[(g, j, ht)][:],
                            start=(ht == 0), stop=False,
                        )
                for j in range(G):
                    # fp8 DoubleRow: contracts h rows 256:512 in one matmul
                    nc.tensor.matmul(
                        py[j][:], w28sb[:, :, dsl], h8[(g, j)][:, 0:2, :],
                        start=False, stop=True,
                        perf_mode=mybir.MatmulPerfMode.DoubleRow,
                    )
                for j in range(G):
                    emit_stt(j, py[j])

            def out_block(g):
                bsl = slice(g * G * TILE, (g + 1) * G * TILE)
                for k in range(2):
                    nc.sync.dma_start(
                        outT_h.ap()[k * 128:(k + 1) * 128, bsl],
                        acc[k][:, bsl],
                    )

            for g in range(NGROUPS + 1):
                if g < NGROUPS:
                    fc1_block(g, 0)
                    fc1_block(g, 1)
                if g > 0:
                    fc2_block(g - 1, 0)
                if g < NGROUPS:
                    fc1_block(g, 2)
                    fc1_block(g, 3)
                if g > 0:
                    fc2_block(g - 1, 1)
                    if g - 1 < NGROUPS - 1:
                        out_block(g - 1)

    nc.compile()
    return nc


_NC = None


def _get_nc():
    global _NC
    if _NC is None:
        _NC = _build()
    return _NC


def make_in_maps(data, w1, b1, w2, b2):
    data = np.asarray(data, dtype=np.float32)
    w1f = np.asarray(w1, dtype=np.float32)
    w2f = np.asarray(w2, dtype=np.float32)
    # packed bf16 weights: w1pk = [w1 rows 0:128 | w1 rows 128:256],
    # w2pk = [w2 rows 0:128 | w2 rows 128:256]
    w1pk = np.ascontiguousarray(np.concatenate(
        [w1f[0:128, :], w1f[128:256, :]], axis=1,
    )).astype(ml_dtypes.bfloat16)
    w2pk = np.ascontiguousarray(np.concatenate(
        [w2f[0:128, :], w2f[128:256, :]], axis=1,
    )).astype(ml_dtypes.bfloat16)
    # DoubleRow pack: [ki, ko, m] = w2[256 + 128*ko + ki, m]
    w28 = np.ascontiguousarray(
        w2f[H // 2:].reshape(2, 128, D).transpose(1, 0, 2)
    ).astype(ml_dtypes.float8_e4m3)
    # packed biases: cols 0:4 = b1 (col ht = rows ht*128..), cols 4:6 = b2
    bpk = np.ascontiguousarray(np.concatenate(
        [np.asarray(b1, dtype=np.float32).reshape(4, 128).T,
         np.asarray(b2, dtype=np.float32).reshape(2, 128).T], axis=1,
    ))

    in_maps = []
    for bb in range(B):
        # residual pre-rolled by +s_t per track:
        # rolled[l, c] = data[(l - s_t) mod L, c]
        rolled = np.empty((L, D), dtype=np.float32)
        for t in range(NT):
            cs = slice(t * TS, (t + 1) * TS)
            rolled[:, cs] = np.roll(data[bb, :, cs], SEFF[t], axis=0)
        for j in range(2):
            sl = slice(j * LC, (j + 1) * LC)
            dataM = np.ascontiguousarray(
                data[bb, sl, :].T.astype(ml_dtypes.bfloat16)
            )
            dataS = np.ascontiguousarray(
                rolled[sl, :].T.astype(ml_dtypes.bfloat16)
            )
            in_maps.append({
                "dataM": dataM, "dataS": dataS,
                "w1pk": w1pk, "w2pk": w2pk, "w28": w28, "bpk": bpk,
            })
    return in_maps


def kernel(data, w1, b1, w2, b2):
    nc = _get_nc()
    in_maps = make_in_maps(data, w1, b1, w2, b2)
    res = bass_utils.run_bass_kernel_spmd(
        nc, in_maps, core_ids=list(range(N_CORES))
    )
    out = np.empty((B, L, D), dtype=np.float32)
    for bb in range(B):
        # full[c, g] = out[(g - s_t(c)) mod L, c]; undo per-track rotation
        full = np.concatenate(
            [np.asarray(res.results[2 * bb + j]["outT"], dtype=np.float32)
             for j in range(2)], axis=1,
        )
        for t in range(NT):
            seg = full[t * TS:(t + 1) * TS, :]
            out[bb, :, t * TS:(t + 1) * TS] = np.roll(seg, -SEFF[t], axis=1).T
    return out
